# revision 6
# baseline (speedup 1.0000x reference)
"""Trainium2 Bass kernel for nn_BatchGraphEncoder (gnn_message_passing).

Math note: the reference's segment softmax uses B unique segment ids
(groups of size 1), so alpha == exp(x-x)/1 == 1.0 bit-exactly for any
finite scores.  The output is therefore independent of the attention
inputs (w_i, w_j, w_k) and reduces to pure batch sums:

    out[:,   0:128] = sum_b h[b,:]      (broadcast over the N=512 rows)
    out[:, 128:256] = sum_b r[b,:]      (broadcast)
    out[:, 256:384] = sum_b t[b,:,:]    ([512, 128])

This is a memory-bound reduction over B=2048 dominated by reading t
(512 MB).  Strategy: shard B across the 8 cores (data parallel), reduce
over the local batch on-device, and sum the 8 tiny partials on the host.

Per-core reduction runs on the VectorEngine (fp32 matmul on the PE is
~4x derated — LOW/HIGH double pass — so the DVE's 1 raw elem/cycle/lane
fold-adds are faster and hide fully under the ~360 GB/s DMA stream).
Tile layout: partition p holds flat columns [512p, 512p+512) of the
[B_loc, 65536] shard; the free dim packs NB batch rows.  In-place
halving folds reduce each tile, then a width-1024 accumulator chain.

The h/r sums ride on the otherwise-idle TensorEngine: a stationary
matrix whose column j is all-ones places column-sums of the moving
operand into PSUM row j (rows 0/1 = sum_h/sum_r).

Load balancing: this machine's core 6 has a slow SDMA engine (E15,
~20.3 vs ~22.9 GB/s), so core 6 gets 228 batch rows while the other
seven get 260.  The kernel is SPMD, so the last 8 tail tiles are
predicated on partition_id != 6 (skipped DMAs over pre-zeroed tiles);
h/r padding rows are zeros, which is exact for a sum.
"""

import numpy as np

B, N, D = 2048, 512, 128
NCORES = 8
FLAT = N * D                 # 65536 flattened (n, d) columns
MMW = 512                    # columns per partition / fold unit

SLOW_CORE = 6
B_FAST = 260                 # rows per fast core
B_SLOW = 228                 # rows for SLOW_CORE
# per-tile batch rows: 14 big tiles then 9 small ones; the last 8 small
# tiles (rows 228..260) are skipped on SLOW_CORE
TILE_PLAN = [16] * 14 + [4] * 9
COND_FROM_ROW = B_SLOW
assert sum(TILE_PLAN) == B_FAST
assert B_FAST * (NCORES - 1) + B_SLOW == B

_BUILT = None
# test.py can inject {"trace": True, ...} here; harness path leaves it empty.
RUN_KWARGS = {}
LAST_RESULTS = None


def _build():
    from concourse import bacc, tile, mybir

    f32 = mybir.dt.float32
    add = mybir.AluOpType.add
    nc = bacc.Bacc(
        "TRN2",
        target_bir_lowering=False,
        debug=False,
        enable_asserts=False,
        num_devices=NCORES,
    )
    t_in = nc.dram_tensor("t_shard", [B_FAST, FLAT], f32, kind="ExternalInput").ap()
    h_in = nc.dram_tensor("h_shard", [B_FAST, D], f32, kind="ExternalInput").ap()
    r_in = nc.dram_tensor("r_shard", [B_FAST, D], f32, kind="ExternalInput").ap()
    out_t = nc.dram_tensor("out_t_part", [128, MMW], f32, kind="ExternalOutput").ap()
    out_hr = nc.dram_tensor("out_hr_part", [2, D], f32, kind="ExternalOutput").ap()

    with tile.TileContext(nc) as tc:
        with (
            tc.tile_pool(name="wconst", bufs=1) as wpool,
            tc.tile_pool(name="loads", bufs=5) as loads,
            tc.tile_pool(name="hr", bufs=6) as hrpool,
            tc.tile_pool(name="res", bufs=1) as res,
            tc.tile_pool(name="acc", bufs=1, space="PSUM") as ppool,
        ):
            # W is zero except column 128 == 1.0; W[:, 128-j : 256-j] is a
            # [128, 128] stationary whose column j is all-ones.
            W = wpool.tile([128, 256], f32)
            nc.vector.memset(W[:], 0.0)
            nc.vector.memset(W[:, 128:129], 1.0)

            psum_hr = ppool.tile([128, D], f32)

            # --- h / r batch sums -> rows 0 / 1 of psum_hr ---
            # (SLOW_CORE's padding rows are zeros; adding them is exact.)
            chunks = []
            for row, src in ((0, h_in), (1, r_in)):
                for c0 in range(0, B_FAST, 128):
                    k = min(128, B_FAST - c0)
                    ht = hrpool.tile([128, D], f32)
                    nc.sync.dma_start(ht[:k, :], src[c0 : c0 + k, :])
                    chunks.append((row, ht, k))
            for i, (row, ht, k) in enumerate(chunks):
                nc.tensor.matmul(
                    psum_hr[:],
                    W[:k, 128 - row : 256 - row],
                    ht[:k, :],
                    start=(i == 0),
                    stop=(i == len(chunks) - 1),
                )

            # --- t batch sum on the DVE ---
            pid_sync = nc.sync.partition_id()
            pid_act = nc.scalar.partition_id()
            cond = {nc.sync: pid_sync != SLOW_CORE, nc.scalar: pid_act != SLOW_CORE}

            acc = res.tile([128, 1024], f32)
            b0 = 0
            for k, NB in enumerate(TILE_PLAN):
                fw = NB * MMW  # free width
                tl = loads.tile([128, 16 * MMW], f32)
                src = t_in[b0 : b0 + NB, :].rearrange("b (p c) -> p b c", p=128)
                conditional = b0 >= COND_FROM_ROW
                b0 += NB
                # Alternate between the SP and ACT HWDGE rings to parallelize
                # descriptor generation (2 KB runs -> ~34 K descriptors).
                dma = nc.sync if k % 2 == 0 else nc.scalar
                dst = tl[:, :fw].rearrange("p (b c) -> p b c", b=NB)
                if conditional:
                    # Skipped on SLOW_CORE: zero the tile first so the fold
                    # adds zeros there.  POOL does the memset; DVE stays free.
                    nc.gpsimd.memset(tl[:, :fw], 0.0)
                    dma.dma_start(dst, src, cond=cond[dma])
                else:
                    dma.dma_start(dst, src)
                half = fw // 2
                while half >= 1024:
                    nc.vector.tensor_tensor(
                        tl[:, :half], tl[:, :half], tl[:, half : 2 * half], add
                    )
                    half //= 2
                if k == 0:
                    nc.vector.tensor_copy(acc[:], tl[:, :1024])
                else:
                    nc.vector.tensor_tensor(acc[:], acc[:], tl[:, :1024], add)

            res_t = res.tile([128, MMW], f32)
            nc.vector.tensor_tensor(res_t[:], acc[:, :512], acc[:, 512:], add)
            nc.sync.dma_start(out_t[:], res_t[:])

            res_hr = res.tile([2, D], f32)
            nc.vector.tensor_copy(res_hr[:], psum_hr[0:2, :])
            nc.sync.dma_start(out_hr[:], res_hr[:])

    nc.compile()
    return nc


def _get_built():
    global _BUILT
    if _BUILT is None:
        _BUILT = _build()
    return _BUILT


def _shard(full, n_rows_used, pad_shape):
    """Contiguous rows -> [B_FAST, ...] buffer, zero-padded past n_rows_used."""
    out = np.zeros(pad_shape, dtype=np.float32)
    out[:n_rows_used] = full
    return out


def kernel(h, r, t, w_i, w_j, w_k):
    global LAST_RESULTS
    from concourse import bass_utils

    nc = _get_built()
    t2 = np.ascontiguousarray(t, dtype=np.float32).reshape(B, FLAT)
    h = np.ascontiguousarray(h, dtype=np.float32)
    r = np.ascontiguousarray(r, dtype=np.float32)

    sizes = [B_FAST] * NCORES
    sizes[SLOW_CORE] = B_SLOW
    starts = np.concatenate([[0], np.cumsum(sizes)])
    in_maps = []
    for c in range(NCORES):
        s, e = int(starts[c]), int(starts[c + 1])
        if e - s == B_FAST:
            in_maps.append(
                {"t_shard": t2[s:e], "h_shard": h[s:e], "r_shard": r[s:e]}
            )
        else:
            in_maps.append(
                {
                    "t_shard": _shard(t2[s:e], e - s, (B_FAST, FLAT)),
                    "h_shard": _shard(h[s:e], e - s, (B_FAST, D)),
                    "r_shard": _shard(r[s:e], e - s, (B_FAST, D)),
                }
            )
    results = bass_utils.run_bass_kernel_spmd(
        nc, in_maps, core_ids=list(range(NCORES)), **RUN_KWARGS
    )
    LAST_RESULTS = results

    sum_t = np.zeros(FLAT, dtype=np.float64)
    sum_h = np.zeros(D, dtype=np.float64)
    sum_r = np.zeros(D, dtype=np.float64)
    for c in range(NCORES):
        sum_t += results.results[c]["out_t_part"].reshape(FLAT)
        sum_h += results.results[c]["out_hr_part"][0]
        sum_r += results.results[c]["out_hr_part"][1]

    out = np.empty((N, 3 * D), dtype=np.float32)
    out[:, 0:D] = sum_h.astype(np.float32)[None, :]
    out[:, D : 2 * D] = sum_r.astype(np.float32)[None, :]
    out[:, 2 * D :] = sum_t.astype(np.float32).reshape(N, D)
    return out
